# revision 15
# baseline (speedup 1.0000x reference)
"""Trainium2 Bass kernel for CustomMultiheadAttention (linear attention with
low-rank QKV projections).

Math (fp32 reference):
    q = elu(query @ q_down_w.T @ q_up_w.T + q_up_b) + 1     # feature map
    k = elu(key   @ k_down_w.T @ k_up_w.T + k_up_b) + 1
    v =      value @ v_down_w.T @ v_up_w.T + v_up_b
    per head h (16 heads, head_dim 64):
        kv_h    = k_h^T v_h                  # [64, 64]
        ksum_h  = sum_t k_h[t]               # [64]
        num     = q_h kv_h                   # [S, 64]
        denom   = q_h . ksum_h               # [S]
        attn_h  = num / (denom + 1e-6)
    out = concat_h(attn_h) @ out_w.T + out_b

Key optimizations over the straightforward 3-stage pipeline:
  * The down/up projections are fused on the host: W_eff = up @ down is
    [E, E]; column-sharding W_eff by head group gives each core a single
    [E, 512] projection (S*E*G flops) instead of a replicated rank-512
    down stage plus a sharded up stage (1.5x the flops).
  * bf16 data path everywhere (inputs, weights, intermediates); PSUM
    accumulation stays fp32. Halves HBM traffic and SBUF pressure.
  * ksum is fused into the kv matmul via a ones-column appended to v.
  * kv accumulates in PSUM across the whole sequence (one accumulation
    group per head-pair bank spanning all 8 chunks).
  * elu+1 = exp(min(u,0)) + max(u,0); exp/relu on the scalar engine with
    fused per-partition bias, min on DVE, final add on gpsimd.
  * reciprocal of the denominator on the scalar engine (fused +1e-6).

Sharding: 8 cores = 4 batches x 2 head-groups (8 heads / 512 embed dims per
core). The output projection is row-sharded; the host adds the two partial
sums plus bias.

Device layouts (per core):
    xq/xk/xv  [E=1024, S=4096] bf16, inputs pre-transposed on host
    k/v proj: stationary = x-chunk e-tile [128e, 128t], moving = W [e, 512j]
              -> feat [t, j] (token partitions), which the kv contraction
              over t needs
    q proj:   stationary = Wq j-tile [e, 128j], moving = x [e, 512t]
              -> qT [j, t] (head-dim partitions), which num/denom need
    kv:       kfeat pair-tile [t, 128] x [v_pair | ones] [t, 130] -> psum
    out:      stationary = attn [j, 128t], moving = wo [j, 512o] -> [t, o]
"""

import numpy as np
import ml_dtypes

import concourse.bass as bass  # noqa: F401
import concourse.mybir as mybir
import concourse.tile as tile
from concourse import bacc
from concourse.bass_utils import run_bass_kernel_spmd

F32 = mybir.dt.float32
BF16 = mybir.dt.bfloat16
AF = mybir.ActivationFunctionType
OP = mybir.AluOpType

P = 128          # partitions
E = 1024         # embed dim
G = 512          # head-group width (8 heads x 64)
S = 4096         # sequence length
B = 4            # batch
TC = 512         # token chunk
NCHUNK = S // TC  # 8
NE = E // P      # 8 e-tiles
NJ = G // P      # 4 j-tiles
NTS = TC // P    # 4 token subtiles per chunk

_CACHE = {}


def _build():
    nc = bacc.Bacc(None, target_bir_lowering=False)

    dp = nc.declare_dram_parameter
    xq = dp("xq", [E, S], BF16, isOutput=False)
    xk = dp("xk", [E, S], BF16, isOutput=False)
    xv = dp("xv", [E, S], BF16, isOutput=False)
    wq = dp("wq", [E, G], BF16, isOutput=False)
    wk = dp("wk", [E, G], BF16, isOutput=False)
    wv = dp("wv", [E, G], BF16, isOutput=False)
    wo = dp("wo", [G, E], BF16, isOutput=False)
    bqt = dp("bqt", [P, NJ], F32, isOutput=False)       # q bias, [128,4] tiles
    bkb = dp("bkb", [P, G], F32, isOutput=False)        # k bias bcast
    bvb = dp("bvb", [P, G], F32, isOutput=False)        # v bias bcast
    rtm = dp("rtm", [P, NJ * 8], F32, isOutput=False)   # head mask tiles
    r8m = dp("r8m", [8, G], F32, isOutput=False)        # head-replication mask
    out_t = dp("out", [S, E], BF16, isOutput=True)

    with tile.TileContext(nc) as tcx:
        from contextlib import ExitStack

        with ExitStack() as root, nc.allow_low_precision(
                reason="bf16 data path; rel tolerance 2e-2"):
            cpool = root.enter_context(tcx.tile_pool(name="consts", bufs=1))
            # weights needed first come first so their DMAs land first
            wk_sb = cpool.tile([P, NE, G], BF16)
            nc.sync.dma_start(
                out=wk_sb[:], in_=wk.rearrange("(a p) j -> p a j", p=P))
            bkb_sb = cpool.tile([P, G], F32)
            nc.sync.dma_start(out=bkb_sb[:], in_=bkb[:])
            wv_sb = cpool.tile([P, NE, G], BF16)
            nc.sync.dma_start(
                out=wv_sb[:], in_=wv.rearrange("(a p) j -> p a j", p=P))
            bvb_sb = cpool.tile([P, G], F32)
            nc.sync.dma_start(out=bvb_sb[:], in_=bvb[:])
            wq_sb = cpool.tile([P, NE, G], BF16)
            nc.sync.dma_start(
                out=wq_sb[:], in_=wq.rearrange("(a p) j -> p a j", p=P))
            bqt_sb = cpool.tile([P, NJ], F32)
            nc.sync.dma_start(out=bqt_sb[:], in_=bqt[:])
            wo_sb = cpool.tile([P, NJ, E], BF16)
            nc.sync.dma_start(
                out=wo_sb[:], in_=wo.rearrange("(a p) o -> p a o", p=P))
            rt_sb = cpool.tile([P, NJ * 8], F32)
            nc.sync.dma_start(out=rt_sb[:], in_=rtm[:])
            r8_sb = cpool.tile([8, G], F32)
            nc.sync.dma_start(out=r8_sb[:], in_=r8m[:])

            kv2 = cpool.tile([P, NJ, P], BF16)    # block-diag kv head pairs
            kblk = cpool.tile([P, NJ * 8], BF16)  # masked ksum for denom
            kvsb = cpool.tile([P, NJ, 130], F32)  # kv psum staging
            qTall = cpool.tile([P, NCHUNK, NJ, TC], BF16)

            # ---------------- Phase A: k/v/q projections + kv accum --------
            with ExitStack() as ph:
                xpool = ph.enter_context(tcx.tile_pool(name="xa", bufs=2))
                fpool = ph.enter_context(tcx.tile_pool(name="fa", bufs=2))
                tpool = ph.enter_context(tcx.tile_pool(name="ta", bufs=2))
                pskvp = ph.enter_context(
                    tcx.tile_pool(name="pskv", bufs=1, space="PSUM"))
                psp = ph.enter_context(
                    tcx.tile_pool(name="psp", bufs=2, space="PSUM"))
                psq = ph.enter_context(
                    tcx.tile_pool(name="psq", bufs=2, space="PSUM"))
                pskv = [pskvp.tile([P, TC], F32, tag=f"kv{j}", name=f"kv{j}")
                        for j in range(NJ)]

                def emit_k(ci):
                    # x loads ride the Activation HW-DGE queue so they don't
                    # serialize behind the weight loads on the SP queue
                    xkt = xpool.tile([P, NE, TC], BF16, tag="xk", name="xkt")
                    nc.scalar.dma_start(
                        out=xkt[:],
                        in_=xk[:, ci * TC:(ci + 1) * TC].rearrange(
                            "(a p) t -> p a t", p=P))
                    kfeat = fpool.tile([P, NTS, G], BF16, tag="kf",
                                       name="kfeat")
                    for ts in range(NTS):
                        pk = psp.tile([P, G], F32, tag="pp", name="pk")
                        for et in range(NE):
                            nc.tensor.matmul(
                                pk[:], xkt[:, et, P * ts:P * (ts + 1)],
                                wk_sb[:, et, :],
                                start=(et == 0), stop=(et == NE - 1))
                        u = tpool.tile([P, G], F32, tag="u", name="u")
                        nc.vector.tensor_tensor(u[:], pk[:], bkb_sb[:],
                                                op=OP.add)
                        m = tpool.tile([P, G], F32, tag="m", name="m")
                        nc.vector.tensor_scalar_min(m[:], u[:], 0.0)
                        ex = tpool.tile([P, G], F32, tag="ex", name="ex")
                        nc.scalar.activation(ex[:], m[:], AF.Exp)
                        nc.vector.scalar_tensor_tensor(
                            kfeat[:, ts, :], u[:], 0.0, ex[:],
                            op0=OP.max, op1=OP.add)
                    return kfeat

                def emit_v(ci):
                    xvt = xpool.tile([P, NE, TC], BF16, tag="xv", name="xvt")
                    nc.scalar.dma_start(
                        out=xvt[:],
                        in_=xv[:, ci * TC:(ci + 1) * TC].rearrange(
                            "(a p) t -> p a t", p=P))
                    vch = fpool.tile([P, NTS, NJ, 132], BF16, tag="vc",
                                     name="vch")
                    nc.gpsimd.memset(vch[:, :, :, P:P + 2], 1.0)
                    for ts in range(NTS):
                        pv = psp.tile([P, G], F32, tag="pp", name="pv")
                        for et in range(NE):
                            nc.tensor.matmul(
                                pv[:], xvt[:, et, P * ts:P * (ts + 1)],
                                wv_sb[:, et, :],
                                start=(et == 0), stop=(et == NE - 1))
                        for j1 in range(NJ):
                            nc.vector.tensor_tensor(
                                vch[:, ts, j1, 0:P],
                                pv[:, P * j1:P * (j1 + 1)],
                                bvb_sb[:, P * j1:P * (j1 + 1)], op=OP.add)
                    return vch

                def emit_q(ci):
                    xqt = xpool.tile([P, NE, TC], BF16, tag="xq", name="xqt")
                    nc.scalar.dma_start(
                        out=xqt[:],
                        in_=xq[:, ci * TC:(ci + 1) * TC].rearrange(
                            "(a p) t -> p a t", p=P))
                    for jt in range(NJ):
                        pq = psq.tile([P, TC], F32, tag="pq", name="pq")
                        for et in range(NE):
                            nc.tensor.matmul(
                                pq[:], wq_sb[:, et, P * jt:P * (jt + 1)],
                                xqt[:, et, :],
                                start=(et == 0), stop=(et == NE - 1))
                        bq_ap = bqt_sb[:, jt:jt + 1]
                        qm = tpool.tile([P, TC], F32, tag="qm", name="qm")
                        nc.vector.tensor_scalar(
                            qm[:], pq[:], bq_ap, 0.0, op0=OP.add, op1=OP.min)
                        qe = tpool.tile([P, TC], F32, tag="qe", name="qe")
                        nc.scalar.activation(qe[:], qm[:], AF.Exp)
                        qu = tpool.tile([P, TC], F32, tag="qu", name="qu")
                        nc.scalar.activation(qu[:], pq[:], AF.Relu,
                                             bias=bq_ap)
                        nc.gpsimd.tensor_tensor(
                            qTall[:, ci, jt, :], qu[:], qe[:], op=OP.add)

                def emit_kv(ci, kfeat, vch):
                    for j1 in range(NJ):
                        for ts in range(NTS):
                            nc.tensor.matmul(
                                pskv[j1][:, 0:130],
                                kfeat[:, ts, P * j1:P * (j1 + 1)],
                                vch[:, ts, j1, 0:130],
                                start=(ci == 0 and ts == 0),
                                stop=(ci == NCHUNK - 1 and ts == NTS - 1))

                for ci in range(NCHUNK):
                    kf = emit_k(ci)
                    vc = emit_v(ci)
                    if ci < NCHUNK - 1:
                        # q before kv: vch evictions finish under the q MMs
                        emit_q(ci)
                        emit_kv(ci, kf, vc)
                    else:
                        # last chunk: kv first so the kv2/kblk build below
                        # overlaps the q projection
                        emit_kv(ci, kf, vc)
                        for j1 in range(NJ):
                            nc.vector.tensor_copy(kvsb[:, j1, :],
                                                  pskv[j1][:, 0:130])
                        emit_q(ci)

            # ---- build block-diag kv2 + masked ksum (kblk) ----
            nc.vector.memset(kv2[:], 0.0)
            for jt in range(NJ):
                h0 = 2 * jt
                nc.vector.tensor_copy(kv2[0:64, jt, 0:64],
                                      kvsb[0:64, jt, 0:64])
                nc.vector.tensor_copy(kv2[64:P, jt, 64:P],
                                      kvsb[64:P, jt, 64:P])
                nc.vector.tensor_scalar(
                    kblk[:, 8 * jt:8 * (jt + 1)],
                    rt_sb[:, 8 * jt:8 * (jt + 1)],
                    kvsb[:, jt, 128:129], None, op0=OP.mult)

            # ---------------- Phase B: attention + output projection -------
            with ExitStack() as ph:
                apool = ph.enter_context(tcx.tile_pool(name="ap", bufs=2))
                rpool = ph.enter_context(tcx.tile_pool(name="rp", bufs=2))
                opool = ph.enter_context(tcx.tile_pool(name="op", bufs=3))
                psd = ph.enter_context(
                    tcx.tile_pool(name="psd", bufs=2, space="PSUM"))
                psn = ph.enter_context(
                    tcx.tile_pool(name="psn", bufs=2, space="PSUM"))
                psr = ph.enter_context(
                    tcx.tile_pool(name="psr", bufs=2, space="PSUM"))
                pso = ph.enter_context(
                    tcx.tile_pool(name="pso", bufs=2, space="PSUM"))

                def emit_attn(ci):
                    pdn = psd.tile([8, TC], F32, tag="pd", name="pdn")
                    for jt in range(NJ):
                        nc.tensor.matmul(
                            pdn[:], kblk[:, 8 * jt:8 * (jt + 1)],
                            qTall[:, ci, jt, :],
                            start=(jt == 0), stop=(jt == NJ - 1))
                    rcp = rpool.tile([8, TC], F32, tag="rcp", name="rcp")
                    nc.vector.reciprocal_approx_fast(out=rcp[:], in_=pdn[:])
                    attn = apool.tile([P, NJ, TC], BF16, tag="at",
                                      name="attn")
                    # rep MMs first (they gate the attn chain), num second
                    prps = []
                    for jt in range(NJ):
                        prp = psr.tile([P, TC], F32, tag="pr", name="prp")
                        nc.tensor.matmul(
                            prp[:], r8_sb[:, P * jt:P * (jt + 1)], rcp[:],
                            start=True, stop=True)
                        rp = rpool.tile([P, TC], F32, tag="rep", name="rep",
                                        bufs=4)
                        nc.scalar.copy(rp[:], prp[:])
                        prps.append(rp)
                    for jt in range(NJ):
                        pnm = psn.tile([P, TC], F32, tag="pn", name="pnm")
                        nc.tensor.matmul(
                            pnm[:], kv2[:, jt, :], qTall[:, ci, jt, :],
                            start=True, stop=True)
                        nc.vector.tensor_tensor(
                            attn[:, jt, :], pnm[:], prps[jt][:], op=OP.mult)
                    return attn

                def emit_out(ci, attn):
                    for ts in range(NTS):
                        ob = opool.tile([P, 2, TC], BF16, tag="ob", name="ob")
                        for oc in range(2):
                            po = pso.tile([P, TC], F32, tag="po", name="po")
                            for jt in range(NJ):
                                nc.tensor.matmul(
                                    po[:], attn[:, jt, P * ts:P * (ts + 1)],
                                    wo_sb[:, jt, TC * oc:TC * (oc + 1)],
                                    start=(jt == 0), stop=(jt == NJ - 1))
                            if oc == 0:
                                nc.scalar.copy(ob[:, oc, :], po[:])
                            else:
                                nc.vector.tensor_copy(ob[:, oc, :], po[:])
                        row0 = ci * TC + ts * P
                        nc.sync.dma_start(
                            out=out_t[row0:row0 + P, :].rearrange(
                                "p (a b) -> p a b", a=2),
                            in_=ob[:])

                # software pipeline: chunk ci's attn chain (recip/rep/mult)
                # resolves while the PE runs chunk ci-1's out projection
                attns = {}
                for ci in range(NCHUNK):
                    attns[ci] = emit_attn(ci)
                    if ci >= 1:
                        emit_out(ci - 1, attns.pop(ci - 1))
                emit_out(NCHUNK - 1, attns.pop(NCHUNK - 1))

    nc.compile()
    return nc


def _get_nc():
    if "nc" not in _CACHE:
        _CACHE["nc"] = _build()
    return _CACHE["nc"]


def _bf16(a):
    return np.ascontiguousarray(np.asarray(a, dtype=np.float32)).astype(
        ml_dtypes.bfloat16)


def kernel(**inputs):
    query = np.asarray(inputs["query"], dtype=np.float32)
    key = np.asarray(inputs["key"], dtype=np.float32)
    value = np.asarray(inputs["value"], dtype=np.float32)

    # host-side fused projection weights (tiny): W_eff = up @ down  [E, E]
    wq_full = np.asarray(inputs["q_up_w"], np.float32) @ np.asarray(
        inputs["q_down_w"], np.float32)
    wk_full = np.asarray(inputs["k_up_w"], np.float32) @ np.asarray(
        inputs["k_down_w"], np.float32)
    wv_full = np.asarray(inputs["v_up_w"], np.float32) @ np.asarray(
        inputs["v_down_w"], np.float32)

    def prep(g):
        gs = slice(G * g, G * (g + 1))
        d = {}
        d["wq"] = _bf16(wq_full[gs].T)                    # [E, G]
        d["wk"] = _bf16(wk_full[gs].T)
        d["wv"] = _bf16(wv_full[gs].T)
        d["wo"] = _bf16(np.asarray(inputs["out_w"], np.float32)[:, gs].T)
        d["bqt"] = np.ascontiguousarray(
            np.asarray(inputs["q_up_b"], np.float32)[gs].reshape(NJ, P).T)
        d["bkb"] = np.ascontiguousarray(np.broadcast_to(
            np.asarray(inputs["k_up_b"], np.float32)[gs], (P, G)))
        d["bvb"] = np.ascontiguousarray(np.broadcast_to(
            np.asarray(inputs["v_up_b"], np.float32)[gs], (P, G)))
        return d

    wg = [prep(0), prep(1)]

    # head masks
    heads = (np.arange(G) // 64)
    rt_full = (heads[:, None] == np.arange(8)[None, :]).astype(np.float32)
    rtm = np.ascontiguousarray(
        rt_full.reshape(NJ, P, 8).transpose(1, 0, 2).reshape(P, NJ * 8))
    r8m = np.ascontiguousarray(rt_full.T)                  # [8, G]

    xT = {}
    for b in range(B):
        xT[("q", b)] = _bf16(query[b].T)
        xT[("k", b)] = _bf16(key[b].T)
        xT[("v", b)] = _bf16(value[b].T)

    in_maps = []
    for c in range(8):
        b, g = divmod(c, 2)
        im = {
            "xq": xT[("q", b)], "xk": xT[("k", b)], "xv": xT[("v", b)],
            "rtm": rtm, "r8m": r8m,
        }
        im.update(wg[g])
        in_maps.append(im)

    nc = _get_nc()
    # the first execution after a device wedge occasionally dies with
    # NRT_EXEC_UNIT_UNRECOVERABLE; a retry on a clean session recovers
    last_err = None
    for _attempt in range(3):
        try:
            res = run_bass_kernel_spmd(nc, in_maps, core_ids=list(range(8)),
                                       **_CACHE.get("run_kwargs", {}))
            last_err = None
            break
        except Exception as e:  # noqa: BLE001
            last_err = e
            import time
            time.sleep(10)
    if last_err is not None:
        raise last_err
    _CACHE["last_result"] = res

    out_b = np.asarray(inputs["out_b"], dtype=np.float32)
    out = np.empty((B, S, E), np.float32)
    for b in range(B):
        out[b] = (res.results[2 * b]["out"].astype(np.float32)
                  + res.results[2 * b + 1]["out"].astype(np.float32)
                  + out_b)
    return out


# revision 23
# speedup vs baseline: 1.1480x; 1.1480x over previous
"""Trainium2 Bass kernel for CustomMultiheadAttention (linear attention with
low-rank QKV projections).

Math (fp32 reference):
    q = elu(query @ q_down_w.T @ q_up_w.T + q_up_b) + 1     # feature map
    k = elu(key   @ k_down_w.T @ k_up_w.T + k_up_b) + 1
    v =      value @ v_down_w.T @ v_up_w.T + v_up_b
    per head h (16 heads, head_dim 64):
        kv_h    = k_h^T v_h                  # [64, 64]
        ksum_h  = sum_t k_h[t]               # [64]
        num     = q_h kv_h                   # [S, 64]
        denom   = q_h . ksum_h               # [S]
        attn_h  = num / (denom + 1e-6)
    out = concat_h(attn_h) @ out_w.T + out_b

Key optimizations over the straightforward 3-stage pipeline:
  * The down/up projections are fused on the host: W_eff = up @ down is
    [E, E]; column-sharding W_eff by head group gives each core a single
    [E, 512] projection (S*E*G flops) instead of a replicated rank-512
    down stage plus a sharded up stage (1.5x the flops).
  * bf16 data path everywhere (inputs, weights, intermediates); PSUM
    accumulation stays fp32. Halves HBM traffic and SBUF pressure.
  * ksum is fused into the kv matmul via a ones-column appended to v.
  * kv accumulates in PSUM across the whole sequence (one accumulation
    group per head-pair bank spanning all 8 chunks).
  * elu+1 = exp(min(u,0)) + max(u,0); exp/relu on the scalar engine with
    fused per-partition bias, min on DVE, final add on gpsimd.
  * reciprocal of the denominator on the scalar engine (fused +1e-6).

Sharding: 8 cores = 4 batches x 2 head-groups (8 heads / 512 embed dims per
core). The output projection is row-sharded; the host adds the two partial
sums plus bias.

Device layouts (per core):
    xq/xk/xv  [E=1024, S=4096] bf16, inputs pre-transposed on host
    k/v proj: stationary = x-chunk e-tile [128e, 128t], moving = W [e, 512j]
              -> feat [t, j] (token partitions), which the kv contraction
              over t needs
    q proj:   stationary = Wq j-tile [e, 128j], moving = x [e, 512t]
              -> qT [j, t] (head-dim partitions), which num/denom need
    kv:       kfeat pair-tile [t, 128] x [v_pair | ones] [t, 130] -> psum
    out:      stationary = attn [j, 128t], moving = wo [j, 512o] -> [t, o]
"""

import numpy as np
import ml_dtypes

import concourse.bass as bass  # noqa: F401
import concourse.mybir as mybir
import concourse.tile as tile
from concourse import bacc
from concourse.bass_utils import run_bass_kernel_spmd

F32 = mybir.dt.float32
BF16 = mybir.dt.bfloat16
AF = mybir.ActivationFunctionType
OP = mybir.AluOpType

P = 128          # partitions
E = 1024         # embed dim
G = 512          # head-group width (8 heads x 64)
S = 4096         # sequence length
B = 4            # batch
TC = 512         # token chunk
NCHUNK = S // TC  # 8
NE = E // P      # 8 e-tiles
NJ = G // P      # 4 j-tiles
NTS = TC // P    # 4 token subtiles per chunk

_CACHE = {}


def _build():
    nc = bacc.Bacc(None, target_bir_lowering=False)

    dp = nc.declare_dram_parameter
    xq = dp("xq", [E, S], BF16, isOutput=False)
    xk = dp("xk", [E, S], BF16, isOutput=False)
    xv = dp("xv", [E, S], BF16, isOutput=False)
    wq = dp("wq", [E, G], BF16, isOutput=False)
    wk = dp("wk", [E, G], BF16, isOutput=False)
    wv = dp("wv", [E, G], BF16, isOutput=False)
    wo = dp("wo", [G, E], BF16, isOutput=False)
    bqt = dp("bqt", [P, NJ], F32, isOutput=False)       # q bias, [128,4] tiles
    bkb = dp("bkb", [P, G], F32, isOutput=False)        # k bias bcast
    bvb = dp("bvb", [P, G], F32, isOutput=False)        # v bias bcast
    rtm = dp("rtm", [P, NJ * 8], F32, isOutput=False)   # head mask tiles
    r8m = dp("r8m", [8, G], BF16, isOutput=False)       # head-replication mask
    out_t = dp("out", [S, E], BF16, isOutput=True)

    with tile.TileContext(nc) as tcx:
        from contextlib import ExitStack

        with ExitStack() as root, nc.allow_low_precision(
                reason="bf16 data path; rel tolerance 2e-2"):
            cpool = root.enter_context(tcx.tile_pool(name="consts", bufs=1))
            # weights needed first come first so their DMAs land first
            wk_sb = cpool.tile([P, NE, G], BF16)
            # first two e-tiles land early so the first matmuls can start
            nc.sync.dma_start(
                out=wk_sb[:, 0:2, :],
                in_=wk[0:2 * P, :].rearrange("(a p) j -> p a j", p=P))
            nc.sync.dma_start(
                out=wk_sb[:, 2:NE, :],
                in_=wk[2 * P:E, :].rearrange("(a p) j -> p a j", p=P))
            bkb_sb = cpool.tile([P, G], F32)
            nc.sync.dma_start(out=bkb_sb[:], in_=bkb[:])
            wv_sb = cpool.tile([P, NE, G], BF16)
            nc.sync.dma_start(
                out=wv_sb[:], in_=wv.rearrange("(a p) j -> p a j", p=P))
            bvb_sb = cpool.tile([P, G], F32)
            nc.sync.dma_start(out=bvb_sb[:], in_=bvb[:])
            wq_sb = cpool.tile([P, NE, G], BF16)
            nc.sync.dma_start(
                out=wq_sb[:], in_=wq.rearrange("(a p) j -> p a j", p=P))
            bqt_sb = cpool.tile([P, NJ], F32)
            nc.sync.dma_start(out=bqt_sb[:], in_=bqt[:])
            wo_sb = cpool.tile([P, NJ, E], BF16)
            nc.sync.dma_start(
                out=wo_sb[:], in_=wo.rearrange("(a p) o -> p a o", p=P))
            rt_sb = cpool.tile([P, NJ * 8], F32)
            nc.sync.dma_start(out=rt_sb[:], in_=rtm[:])
            r8_sb = cpool.tile([8, G], BF16)
            nc.sync.dma_start(out=r8_sb[:], in_=r8m[:])

            kv2 = cpool.tile([P, NJ, P], BF16)    # block-diag kv head pairs
            kblk = cpool.tile([P, NJ * 8], BF16)  # masked ksum for denom
            kvsb = cpool.tile([P, NJ, 130], F32)  # kv psum staging
            qTall = cpool.tile([P, NCHUNK, NJ, TC], BF16)

            # ---------------- Phase A: k/v/q projections + kv accum --------
            with ExitStack() as ph:
                xpool = ph.enter_context(tcx.tile_pool(name="xa", bufs=2))
                fpool = ph.enter_context(tcx.tile_pool(name="fa", bufs=2))
                tpool = ph.enter_context(tcx.tile_pool(name="ta", bufs=2))
                pskvp = ph.enter_context(
                    tcx.tile_pool(name="pskv", bufs=1, space="PSUM"))
                psp = ph.enter_context(
                    tcx.tile_pool(name="psp", bufs=2, space="PSUM"))
                psq = ph.enter_context(
                    tcx.tile_pool(name="psq", bufs=2, space="PSUM"))
                pskv = [pskvp.tile([P, TC], F32, tag=f"kv{j}", name=f"kv{j}")
                        for j in range(NJ)]

                def emit_k(ci):
                    # x loads ride the Activation HW-DGE queue so they don't
                    # serialize behind the weight loads on the SP queue
                    xkt = xpool.tile([P, NE, TC], BF16, tag="xk", name="xkt")
                    if ci == 0:
                        # split so the first token subtile lands early
                        nc.scalar.dma_start(
                            out=xkt[:, :, 0:P],
                            in_=xk[:, 0:P].rearrange("(a p) t -> p a t", p=P))
                        nc.scalar.dma_start(
                            out=xkt[:, :, P:TC],
                            in_=xk[:, P:TC].rearrange("(a p) t -> p a t",
                                                      p=P))
                    else:
                        nc.scalar.dma_start(
                            out=xkt[:],
                            in_=xk[:, ci * TC:(ci + 1) * TC].rearrange(
                                "(a p) t -> p a t", p=P))
                    kfeat = fpool.tile([P, NTS, G], BF16, tag="kf",
                                       name="kfeat")
                    for ts in range(NTS):
                        pk = psp.tile([P, G], F32, tag="pp", name="pk")
                        for et in range(NE):
                            nc.tensor.matmul(
                                pk[:], xkt[:, et, P * ts:P * (ts + 1)],
                                wk_sb[:, et, :],
                                start=(et == 0), stop=(et == NE - 1))
                        u = tpool.tile([P, G], F32, tag="u", name="u")
                        nc.vector.tensor_tensor(u[:], pk[:], bkb_sb[:],
                                                op=OP.add)
                        m = tpool.tile([P, G], F32, tag="m", name="m")
                        nc.vector.tensor_scalar_min(m[:], u[:], 0.0)
                        ex = tpool.tile([P, G], F32, tag="ex", name="ex")
                        nc.scalar.activation(ex[:], m[:], AF.Exp)
                        nc.vector.scalar_tensor_tensor(
                            kfeat[:, ts, :], u[:], 0.0, ex[:],
                            op0=OP.max, op1=OP.add)
                    return kfeat

                def emit_v(ci):
                    xvt = xpool.tile([P, NE, TC], BF16, tag="xv", name="xvt")
                    nc.scalar.dma_start(
                        out=xvt[:],
                        in_=xv[:, ci * TC:(ci + 1) * TC].rearrange(
                            "(a p) t -> p a t", p=P))
                    vch = fpool.tile([P, NTS, NJ, 132], BF16, tag="vc",
                                     name="vch")
                    nc.gpsimd.memset(vch[:, :, :, P:P + 2], 1.0)
                    for ts in range(NTS):
                        pv = psp.tile([P, G], F32, tag="pp", name="pv")
                        for et in range(NE):
                            nc.tensor.matmul(
                                pv[:], xvt[:, et, P * ts:P * (ts + 1)],
                                wv_sb[:, et, :],
                                start=(et == 0), stop=(et == NE - 1))
                        for j1 in range(NJ):
                            nc.vector.tensor_tensor(
                                vch[:, ts, j1, 0:P],
                                pv[:, P * j1:P * (j1 + 1)],
                                bvb_sb[:, P * j1:P * (j1 + 1)], op=OP.add)
                    return vch

                def emit_q(ci):
                    xqt = xpool.tile([P, NE, TC], BF16, tag="xq", name="xqt")
                    nc.scalar.dma_start(
                        out=xqt[:],
                        in_=xq[:, ci * TC:(ci + 1) * TC].rearrange(
                            "(a p) t -> p a t", p=P))
                    for jt in range(NJ):
                        pq = psq.tile([P, TC], F32, tag="pq", name="pq")
                        for et in range(NE):
                            nc.tensor.matmul(
                                pq[:], wq_sb[:, et, P * jt:P * (jt + 1)],
                                xqt[:, et, :],
                                start=(et == 0), stop=(et == NE - 1))
                        bq_ap = bqt_sb[:, jt:jt + 1]
                        qm = tpool.tile([P, TC], F32, tag="qm", name="qm")
                        nc.vector.tensor_scalar(
                            qm[:], pq[:], bq_ap, 0.0, op0=OP.add, op1=OP.min)
                        qe = tpool.tile([P, TC], F32, tag="qe", name="qe")
                        nc.scalar.activation(qe[:], qm[:], AF.Exp)
                        qu = tpool.tile([P, TC], F32, tag="qu", name="qu")
                        nc.scalar.activation(qu[:], pq[:], AF.Relu,
                                             bias=bq_ap)
                        nc.gpsimd.tensor_tensor(
                            qTall[:, ci, jt, :], qu[:], qe[:], op=OP.add)

                def emit_kv(ci, kfeat, vch):
                    for j1 in range(NJ):
                        for ts in range(NTS):
                            nc.tensor.matmul(
                                pskv[j1][:, 0:130],
                                kfeat[:, ts, P * j1:P * (j1 + 1)],
                                vch[:, ts, j1, 0:130],
                                start=(ci == 0 and ts == 0),
                                stop=(ci == NCHUNK - 1 and ts == NTS - 1))

                for ci in range(NCHUNK):
                    kf = emit_k(ci)
                    vc = emit_v(ci)
                    if ci < NCHUNK - 1:
                        # q before kv: vch evictions finish under the q MMs
                        emit_q(ci)
                        emit_kv(ci, kf, vc)
                    else:
                        # last chunk: kv first so the kv2/kblk build below
                        # overlaps the q projection
                        emit_kv(ci, kf, vc)
                        for j1 in range(NJ):
                            nc.vector.tensor_copy(kvsb[:, j1, :],
                                                  pskv[j1][:, 0:130])
                        emit_q(ci)

            # ---- build block-diag kv2 + masked ksum (kblk) ----
            nc.vector.memset(kv2[:], 0.0)
            for jt in range(NJ):
                h0 = 2 * jt
                nc.vector.tensor_copy(kv2[0:64, jt, 0:64],
                                      kvsb[0:64, jt, 0:64])
                nc.vector.tensor_copy(kv2[64:P, jt, 64:P],
                                      kvsb[64:P, jt, 64:P])
                nc.vector.tensor_scalar(
                    kblk[:, 8 * jt:8 * (jt + 1)],
                    rt_sb[:, 8 * jt:8 * (jt + 1)],
                    kvsb[:, jt, 128:129], None, op0=OP.mult)

            # ---------------- Phase B: attention + output projection -------
            with ExitStack() as ph:
                apool = ph.enter_context(tcx.tile_pool(name="ap", bufs=2))
                rpool = ph.enter_context(tcx.tile_pool(name="rp", bufs=2))
                opool = ph.enter_context(tcx.tile_pool(name="op", bufs=3))
                psd = ph.enter_context(
                    tcx.tile_pool(name="psd", bufs=1, space="PSUM"))
                psn = ph.enter_context(
                    tcx.tile_pool(name="psn", bufs=2, space="PSUM"))
                psr = ph.enter_context(
                    tcx.tile_pool(name="psr", bufs=2, space="PSUM"))
                pso = ph.enter_context(
                    tcx.tile_pool(name="pso", bufs=3, space="PSUM"))

                def emit_attn(ci):
                    pdn = psd.tile([8, TC], F32, tag="pd", name="pdn")
                    for jt in range(NJ):
                        nc.tensor.matmul(
                            pdn[:], kblk[:, 8 * jt:8 * (jt + 1)],
                            qTall[:, ci, jt, :],
                            start=(jt == 0), stop=(jt == NJ - 1))
                    rcf = rpool.tile([8, TC], F32, tag="rcf", name="rcf")
                    nc.vector.reciprocal_approx_fast(out=rcf[:], in_=pdn[:])
                    # bf16 so the rep matmuls stream at full rate (fp32
                    # moving operands run the PE at half speed)
                    rcp = rpool.tile([8, TC], BF16, tag="rcp", name="rcp")
                    nc.vector.tensor_copy(rcp[:], rcf[:])
                    attn = apool.tile([P, NJ, TC], BF16, tag="at",
                                      name="attn")
                    # rep MMs first (they gate the attn chain), num second
                    prps = []
                    for jt in range(NJ):
                        prp = psr.tile([P, TC], F32, tag="pr", name="prp")
                        nc.tensor.matmul(
                            prp[:], r8_sb[:, P * jt:P * (jt + 1)], rcp[:],
                            start=True, stop=True)
                        rp = rpool.tile([P, TC], F32, tag="rep", name="rep",
                                        bufs=4)
                        nc.scalar.copy(rp[:], prp[:])
                        prps.append(rp)
                    for jt in range(NJ):
                        pnm = psn.tile([P, TC], F32, tag="pn", name="pnm")
                        nc.tensor.matmul(
                            pnm[:], kv2[:, jt, :], qTall[:, ci, jt, :],
                            start=True, stop=True)
                        nc.vector.tensor_tensor(
                            attn[:, jt, :], pnm[:], prps[jt][:], op=OP.mult)
                    return attn

                def emit_out(ci, attn):
                    # one 1MB DMA per chunk keeps the write queue from
                    # backpressuring the PSUM evictions; the last chunk
                    # writes per-ts so the kernel tail stays short
                    split = (ci == NCHUNK - 1)
                    ob = opool.tile([P, NTS, 2, TC], BF16, tag="ob",
                                    name="ob")
                    for ts in range(NTS):
                        for oc in range(2):
                            po = pso.tile([P, TC], F32, tag="po", name="po")
                            for jt in range(NJ):
                                nc.tensor.matmul(
                                    po[:], attn[:, jt, P * ts:P * (ts + 1)],
                                    wo_sb[:, jt, TC * oc:TC * (oc + 1)],
                                    start=(jt == 0), stop=(jt == NJ - 1))
                            if oc == 0:
                                nc.scalar.copy(ob[:, ts, oc, :], po[:])
                            else:
                                nc.vector.tensor_copy(ob[:, ts, oc, :],
                                                      po[:])
                        if split:
                            row0 = ci * TC + ts * P
                            nc.sync.dma_start(
                                out=out_t[row0:row0 + P, :].rearrange(
                                    "p (c b) -> p c b", c=2),
                                in_=ob[:, ts, :, :])
                    if not split:
                        nc.sync.dma_start(
                            out=out_t[ci * TC:(ci + 1) * TC, :].rearrange(
                                "(a p) (c b) -> p a c b", p=P, c=2),
                            in_=ob[:])

                # software pipeline: chunk ci's attn chain (recip/rep/mult)
                # resolves while the PE runs chunk ci-1's out projection
                attns = {}
                for ci in range(NCHUNK):
                    attns[ci] = emit_attn(ci)
                    if ci >= 1:
                        emit_out(ci - 1, attns.pop(ci - 1))
                emit_out(NCHUNK - 1, attns.pop(NCHUNK - 1))

    nc.compile()
    return nc


def _get_nc():
    if "nc" not in _CACHE:
        _CACHE["nc"] = _build()
    return _CACHE["nc"]


def _bf16(a):
    return np.ascontiguousarray(np.asarray(a, dtype=np.float32)).astype(
        ml_dtypes.bfloat16)


def kernel(**inputs):
    query = np.asarray(inputs["query"], dtype=np.float32)
    key = np.asarray(inputs["key"], dtype=np.float32)
    value = np.asarray(inputs["value"], dtype=np.float32)

    # host-side fused projection weights (tiny): W_eff = up @ down  [E, E]
    wq_full = np.asarray(inputs["q_up_w"], np.float32) @ np.asarray(
        inputs["q_down_w"], np.float32)
    wk_full = np.asarray(inputs["k_up_w"], np.float32) @ np.asarray(
        inputs["k_down_w"], np.float32)
    wv_full = np.asarray(inputs["v_up_w"], np.float32) @ np.asarray(
        inputs["v_down_w"], np.float32)

    def prep(g):
        gs = slice(G * g, G * (g + 1))
        d = {}
        d["wq"] = _bf16(wq_full[gs].T)                    # [E, G]
        d["wk"] = _bf16(wk_full[gs].T)
        d["wv"] = _bf16(wv_full[gs].T)
        d["wo"] = _bf16(np.asarray(inputs["out_w"], np.float32)[:, gs].T)
        d["bqt"] = np.ascontiguousarray(
            np.asarray(inputs["q_up_b"], np.float32)[gs].reshape(NJ, P).T)
        d["bkb"] = np.ascontiguousarray(np.broadcast_to(
            np.asarray(inputs["k_up_b"], np.float32)[gs], (P, G)))
        d["bvb"] = np.ascontiguousarray(np.broadcast_to(
            np.asarray(inputs["v_up_b"], np.float32)[gs], (P, G)))
        return d

    wg = [prep(0), prep(1)]

    # head masks
    heads = (np.arange(G) // 64)
    rt_full = (heads[:, None] == np.arange(8)[None, :]).astype(np.float32)
    rtm = np.ascontiguousarray(
        rt_full.reshape(NJ, P, 8).transpose(1, 0, 2).reshape(P, NJ * 8))
    r8m = _bf16(rt_full.T)                                 # [8, G]

    xT = {}
    for b in range(B):
        xT[("q", b)] = _bf16(query[b].T)
        xT[("k", b)] = _bf16(key[b].T)
        xT[("v", b)] = _bf16(value[b].T)

    in_maps = []
    for c in range(8):
        b, g = divmod(c, 2)
        im = {
            "xq": xT[("q", b)], "xk": xT[("k", b)], "xv": xT[("v", b)],
            "rtm": rtm, "r8m": r8m,
        }
        im.update(wg[g])
        in_maps.append(im)

    nc = _get_nc()
    # the first execution after a device wedge occasionally dies with
    # NRT_EXEC_UNIT_UNRECOVERABLE; a retry on a clean session recovers
    last_err = None
    for _attempt in range(3):
        try:
            res = run_bass_kernel_spmd(nc, in_maps, core_ids=list(range(8)),
                                       **_CACHE.get("run_kwargs", {}))
            last_err = None
            break
        except Exception as e:  # noqa: BLE001
            last_err = e
            import time
            time.sleep(10)
    if last_err is not None:
        raise last_err
    _CACHE["last_result"] = res

    out_b = np.asarray(inputs["out_b"], dtype=np.float32)
    out = np.empty((B, S, E), np.float32)
    for b in range(B):
        out[b] = (res.results[2 * b]["out"].astype(np.float32)
                  + res.results[2 * b + 1]["out"].astype(np.float32)
                  + out_b)
    return out
